# Initial kernel scaffold
#
import sys
import time

sys.path.insert(0, "/opt/trn_rl_repo")

import numpy as np
import bass_rust
import concourse.bass as bass
import concourse.mybir as mybir
from concourse import tile, masks
from concourse.bass_utils import run_bass_kernel_spmd

F32 = mybir.dt.float32
NHEADS = 5
D = 128
NB = 2
SEQ = 1024
TOK = NB * SEQ
H = 5120
NFT = 15
NHT = H // 128


def split_multi_waits(nc, max_waits=1):
    n = 0
    for fn in nc.m.functions:
        for bb in fn.blocks:
            out = []
            for inst in bb.instructions:
                si = inst.sync_info
                if si is not None and si.on_wait and len(si.on_wait) > max_waits:
                    waits = list(si.on_wait)
                    extra, keep = waits[:-max_waits], waits[-max_waits:]
                    for k, w in enumerate(extra):
                        nop = bass_rust.InstNoOp(name=f"{inst.name}_sw{k}")
                        nop.engine = inst.engine
                        nop.sync_info = mybir.SyncInfo(on_wait=[w], on_update=[])
                        out.append(nop)
                        n += 1
                    inst.sync_info = mybir.SyncInfo(
                        on_wait=keep, on_update=list(si.on_update))
                out.append(inst)
            bb.instructions = out
    return n


def build_nc():
    from contextlib import ExitStack

    nc = bass.Bass()
    hidT = nc.dram_tensor("hidT", [H, TOK], F32, kind="ExternalInput")
    wqkvT = nc.dram_tensor("wqkvT", [H, NFT * 128], F32, kind="ExternalInput")
    oT = nc.dram_tensor("oT", [NHEADS * 128, H], F32, kind="ExternalInput")
    slopej = nc.dram_tensor("slopej", [NHEADS, 128, 512], F32, kind="ExternalInput")
    rowmask = nc.dram_tensor("rowmask", [NHEADS, 8, 128, 512], F32,
                             kind="ExternalInput")
    expb = nc.dram_tensor("expb", [NHEADS, 128, 8], F32, kind="ExternalInput")
    out = nc.dram_tensor("out", [TOK, H], F32, kind="ExternalOutput")
    attnT_d = nc.dram_tensor("attnT_scratch", [NHEADS * 128, TOK], F32)

    dma = nc.sync

    with tile.TileContext(nc) as tc:
        with ExitStack() as octx:
            const_pool = octx.enter_context(tc.tile_pool(name="const", bufs=1))
            ident = const_pool.tile([128, 128], F32)
            masks.make_identity(nc, ident[:])

            with ExitStack() as main:
                hid_pool = main.enter_context(tc.tile_pool(name="hid", bufs=42))
                w_pool = main.enter_context(tc.tile_pool(name="wqkv", bufs=2))
                qt_pool = main.enter_context(tc.tile_pool(name="qT", bufs=2))
                kt_pool = main.enter_context(tc.tile_pool(name="kT", bufs=2))
                v_pool = main.enter_context(tc.tile_pool(name="v", bufs=2))
                vt_pool = main.enter_context(tc.tile_pool(name="vT", bufs=3))
                sp_pool = main.enter_context(tc.tile_pool(name="sP", bufs=2))
                pt_pool = main.enter_context(tc.tile_pool(name="pT", bufs=2))
                msk_pool = main.enter_context(tc.tile_pool(name="msk", bufs=3))
                sml_pool = main.enter_context(tc.tile_pool(name="sml", bufs=4))
                ats_pool = main.enter_context(tc.tile_pool(name="ats", bufs=3))

                ps512 = main.enter_context(
                    tc.tile_pool(name="ps512", bufs=2, space="PSUM"))
                pstr = main.enter_context(
                    tc.tile_pool(name="pstr", bufs=2, space="PSUM"))
                psS = main.enter_context(
                    tc.tile_pool(name="psS", bufs=1, space="PSUM"))
                psA = main.enter_context(
                    tc.tile_pool(name="psA", bufs=2, space="PSUM"))

                kT = {}
                vt = {}

                for qtr in range(4):
                    b, lh = qtr // 2, qtr % 2
                    tq = qtr * 512

                    hid_t = []
                    for ht in range(NHT):
                        t = hid_pool.tile([128, 512], F32, tag="hid")
                        dma.dma_start(
                            t[:], hidT[ht * 128:(ht + 1) * 128, tq:tq + 512])
                        hid_t.append(t)

                    if lh == 0:
                        for h in range(NHEADS):
                            kT[(b, h)] = kt_pool.tile([128, SEQ], F32,
                                                      tag=f"kT{h}")
                        for kt in range(8):
                            vt[(b, kt)] = v_pool.tile([128, NHEADS * 128], F32,
                                                      tag=f"v{kt}")

                    qT = {}
                    for ft in range(NFT):
                        ps = ps512.tile([128, 512], F32, tag="ps512")
                        for hg in range(4):
                            wt = w_pool.tile([128, 1280], F32, tag="w")
                            src = wqkvT[hg * 1280:(hg + 1) * 1280,
                                        ft * 128:(ft + 1) * 128]
                            dma.dma_start(
                                wt[:].rearrange("p (g c) -> p g c", c=128),
                                src.rearrange("(g p) c -> p g c", p=128))
                            for hl in range(10):
                                ht = hg * 10 + hl
                                nc.tensor.matmul(
                                    ps[:], wt[:, hl * 128:(hl + 1) * 128],
                                    hid_t[ht][:],
                                    start=(ht == 0), stop=(ht == NHT - 1))
                        if ft < 5:
                            h = ft
                            q = qt_pool.tile([128, 512], F32, tag=f"qT{h}")
                            nc.vector.tensor_copy(q[:], ps[:])
                            qT[h] = q
                        elif ft < 10:
                            h = ft - 5
                            nc.vector.tensor_copy(
                                kT[(b, h)][:, lh * 512:(lh + 1) * 512], ps[:])
                        else:
                            h = ft - 10
                            vtt = vt_pool.tile([128, 512], F32, tag="vT")
                            nc.vector.tensor_copy(vtt[:], ps[:])
                            for i in range(4):
                                ktl = lh * 4 + i
                                pt = pstr.tile([128, 128], F32, tag="pstr")
                                nc.tensor.transpose(
                                    pt[:], vtt[:, i * 128:(i + 1) * 128],
                                    ident[:])
                                nc.vector.tensor_copy(
                                    vt[(b, ktl)][:, h * 128:(h + 1) * 128],
                                    pt[:])

                    for h in range(NHEADS):
                        eb = sml_pool.tile([128, 8], F32, tag="eb")
                        dma.dma_start(eb[:], expb[h])
                        sj = None
                        if lh == 1:
                            sj = msk_pool.tile([128, 512], F32, tag="sj")
                            dma.dma_start(sj[:], slopej[h])
                        for i in range(4):
                            qtg = lh * 4 + i
                            T = (qtg + 1) * 128
                            nfull = (qtg * 128) // 512
                            off = nfull * 512
                            W = T - off

                            rm = msk_pool.tile([128, 512], F32, tag="rm")
                            dma.dma_start(rm[:, 0:W], rowmask[h, qtg, :, 0:W])

                            psc = psS.tile([128, 1024], F32, tag="psS")
                            if nfull:
                                nc.tensor.matmul(
                                    psc[:, 0:512],
                                    qT[h][:, i * 128:(i + 1) * 128],
                                    kT[(b, h)][:, 0:512],
                                    start=True, stop=True)
                            nc.tensor.matmul(
                                psc[:, off:off + W],
                                qT[h][:, i * 128:(i + 1) * 128],
                                kT[(b, h)][:, off:off + W],
                                start=True, stop=True)

                            sp = sp_pool.tile([128, 1024], F32, tag="sP")
                            if nfull:
                                nc.vector.tensor_add(
                                    sp[:, 0:512], psc[:, 0:512], sj[:])
                            nc.vector.tensor_add(
                                sp[:, off:off + W], psc[:, off:off + W],
                                rm[:, 0:W])

                            rs = sml_pool.tile([128, 1], F32, tag="rs")
                            nc.scalar.activation(
                                sp[:, 0:T], sp[:, 0:T],
                                mybir.ActivationFunctionType.Exp,
                                bias=eb[:, qtg:qtg + 1], scale=1.0,
                                accum_out=rs[:])
                            rc = sml_pool.tile([128, 1], F32, tag="rc")
                            nc.vector.reciprocal(rc[:], rs[:])
                            nc.vector.tensor_scalar_mul(
                                sp[:, 0:T], sp[:, 0:T], rc[:])

                            ptt = pt_pool.tile([128, 1024], F32, tag="pT")
                            for kt in range(qtg + 1):
                                pp = pstr.tile([128, 128], F32, tag="pstr")
                                nc.tensor.transpose(
                                    pp[:], sp[:, kt * 128:(kt + 1) * 128],
                                    ident[:])
                                nc.vector.tensor_copy(
                                    ptt[:, kt * 128:(kt + 1) * 128], pp[:])

                            pa = psA.tile([128, 128], F32, tag="psA")
                            for kt in range(qtg + 1):
                                nc.tensor.matmul(
                                    pa[:],
                                    vt[(b, kt)][:, h * 128:(h + 1) * 128],
                                    ptt[:, kt * 128:(kt + 1) * 128],
                                    start=(kt == 0), stop=(kt == qtg))
                            ats = ats_pool.tile([128, 128], F32, tag="ats")
                            nc.vector.tensor_copy(ats[:], pa[:])
                            dma.dma_start(
                                attnT_d[h * 128:(h + 1) * 128,
                                        b * SEQ + qtg * 128:
                                        b * SEQ + (qtg + 1) * 128],
                                ats[:])

            with ExitStack() as tail:
                at_pool = tail.enter_context(tc.tile_pool(name="aT", bufs=6))
                ot_pool = tail.enter_context(tc.tile_pool(name="oT", bufs=7))
                oe_pool = tail.enter_context(tc.tile_pool(name="oev", bufs=3))
                pso = tail.enter_context(
                    tc.tile_pool(name="psO", bufs=2, space="PSUM"))

                for tch in range(2):
                    aT = []
                    for fh in range(NHEADS):
                        a = at_pool.tile([128, 1024], F32, tag=f"aT{fh}")
                        dma.dma_start(
                            a[:], attnT_d[fh * 128:(fh + 1) * 128,
                                          tch * 1024:(tch + 1) * 1024])
                        aT.append(a)
                    for oc in range(10):
                        ott = []
                        for fh in range(NHEADS):
                            o = ot_pool.tile([128, 512], F32, tag="oT")
                            dma.dma_start(
                                o[:], oT[fh * 128:(fh + 1) * 128,
                                         oc * 512:(oc + 1) * 512])
                            ott.append(o)
                        for tt in range(8):
                            ps = pso.tile([128, 512], F32, tag="psO")
                            for fh in range(NHEADS):
                                nc.tensor.matmul(
                                    ps[:], aT[fh][:, tt * 128:(tt + 1) * 128],
                                    ott[fh][:],
                                    start=(fh == 0), stop=(fh == NHEADS - 1))
                            oe = oe_pool.tile([128, 512], F32, tag="oev")
                            nc.vector.tensor_copy(oe[:], ps[:])
                            dma.dma_start(
                                out[tch * 1024 + tt * 128:
                                    tch * 1024 + (tt + 1) * 128,
                                    oc * 512:(oc + 1) * 512],
                                oe[:])

    split_multi_waits(nc)
    return nc


_NC = None


def _get_nc():
    global _NC
    if _NC is None:
        _NC = build_nc()
    return _NC


def prep_inputs(hidden_states, attention_mask, W_pack, o_proj):
    hs = np.asarray(hidden_states, dtype=np.float32)
    mask = np.asarray(attention_mask, dtype=np.float32)
    wp = np.asarray(W_pack, dtype=np.float32)
    op = np.asarray(o_proj, dtype=np.float32)

    hidT = np.ascontiguousarray(
        hs.reshape(TOK, H).T)
    scale = 1.0 / np.sqrt(D)
    in_maps = []
    for c in range(8):
        heads = range(5 * c, 5 * c + 5)
        wcols = []
        for kind, base in (("q", 0), ("k", H), ("v", 2 * H)):
            for g in heads:
                blk = wp[base + g * D: base + (g + 1) * D, :]
                if kind == "q":
                    blk = blk * scale
                wcols.append(blk.T)
        wqkvT = np.ascontiguousarray(np.concatenate(wcols, axis=1))
        oTc = np.ascontiguousarray(op[:, 640 * c: 640 * (c + 1)].T)

        sj = np.empty((NHEADS, 128, 512), dtype=np.float32)
        rmk = np.full((NHEADS, 8, 128, 512), -1e30, dtype=np.float32)
        eb = np.empty((NHEADS, 128, 8), dtype=np.float32)
        for hl, g in enumerate(heads):
            m = mask[g]
            sj[hl, :, :] = m[1023, :512][None, :]
            diag = np.diagonal(m).astype(np.float32)
            for qt in range(8):
                Wd = (qt % 4 + 1) * 128
                off = (qt // 4) * 512
                rmk[hl, qt, :, :Wd] = np.maximum(
                    m[qt * 128:(qt + 1) * 128, off:off + Wd], -1e30)
                eb[hl, :, qt] = -(diag[qt * 128:(qt + 1) * 128] + 30.0)
        in_maps.append({
            "hidT": hidT, "wqkvT": wqkvT, "oT": oTc,
            "slopej": sj, "rowmask": rmk, "expb": eb,
        })
    return in_maps


def run(in_maps):
    nc = _get_nc()
    res = run_bass_kernel_spmd(nc, in_maps, core_ids=list(range(8)))
    return res


def kernel(hidden_states, attention_mask, W_pack, o_proj):
    in_maps = prep_inputs(hidden_states, attention_mask, W_pack, o_proj)
    res = run(in_maps)
    total = np.zeros((TOK, H), dtype=np.float32)
    for c in range(8):
        total += res.results[c]["out"]
    return total.reshape(NB, SEQ, H)


if __name__ == "__main__":
    t0 = time.time()
    nc = _get_nc()
    print("build+schedule ok in", time.time() - t0, "s")


# revision 10
# speedup vs baseline: 445.9797x; 445.9797x over previous
import sys
import time

sys.path.insert(0, "/opt/trn_rl_repo")

import numpy as np
import bass_rust
import concourse.bass as bass
import concourse.mybir as mybir
from concourse import tile, masks
from concourse.bass_utils import run_bass_kernel_spmd

F32 = mybir.dt.float32
NHEADS = 5
D = 128
NB = 2
SEQ = 1024
TOK = NB * SEQ
H = 5120
NFT = 15
NHT = H // 128


def split_multi_waits(nc, max_waits=1):
    n = 0
    for fn in nc.m.functions:
        for bb in fn.blocks:
            out = []
            for inst in bb.instructions:
                si = inst.sync_info
                if si is not None and si.on_wait and len(si.on_wait) > max_waits:
                    waits = list(si.on_wait)
                    extra, keep = waits[:-max_waits], waits[-max_waits:]
                    for k, w in enumerate(extra):
                        nop = bass_rust.InstNoOp(name=f"{inst.name}_sw{k}")
                        nop.engine = inst.engine
                        nop.sync_info = mybir.SyncInfo(on_wait=[w], on_update=[])
                        out.append(nop)
                        n += 1
                    inst.sync_info = mybir.SyncInfo(
                        on_wait=keep, on_update=list(si.on_update))
                out.append(inst)
            bb.instructions = out
    return n


def build_nc():
    from contextlib import ExitStack

    nc = bass.Bass()
    hidT = nc.dram_tensor("hidT", [H, TOK], F32, kind="ExternalInput")
    wqkvT = nc.dram_tensor("wqkvT", [H, NFT * 128], F32, kind="ExternalInput")
    oT = nc.dram_tensor("oT", [NHEADS * 128, H], F32, kind="ExternalInput")
    slopej = nc.dram_tensor("slopej", [NHEADS, 128, 512], F32, kind="ExternalInput")
    rowmask = nc.dram_tensor("rowmask", [NHEADS, 8, 128, 512], F32,
                             kind="ExternalInput")
    expb = nc.dram_tensor("expb", [NHEADS, 128, 8], F32, kind="ExternalInput")
    out = nc.dram_tensor("out", [TOK, H], F32, kind="ExternalOutput")
    attnT_d = nc.dram_tensor("attnT_scratch", [NHEADS * 128, TOK], F32)

    dma = nc.sync

    with tile.TileContext(nc) as tc:
        with ExitStack() as octx:
            const_pool = octx.enter_context(tc.tile_pool(name="const", bufs=1))
            ident = const_pool.tile([128, 128], F32)
            masks.make_identity(nc, ident[:])

            with ExitStack() as main:
                hid_pool = main.enter_context(tc.tile_pool(name="hid", bufs=40))
                w_pool = main.enter_context(tc.tile_pool(name="wqkv", bufs=2))
                qt_pool = main.enter_context(tc.tile_pool(name="qT", bufs=2))
                kt_pool = main.enter_context(tc.tile_pool(name="kT", bufs=2))
                v_pool = main.enter_context(tc.tile_pool(name="v", bufs=1))
                vt_pool = main.enter_context(tc.tile_pool(name="vT", bufs=2))
                sp_pool = main.enter_context(tc.tile_pool(name="sP", bufs=2))
                pt_pool = main.enter_context(tc.tile_pool(name="pT", bufs=2))
                msk_pool = main.enter_context(tc.tile_pool(name="msk", bufs=2))
                sml_pool = main.enter_context(tc.tile_pool(name="sml", bufs=4))
                ats_pool = main.enter_context(tc.tile_pool(name="ats", bufs=2))

                ps512 = main.enter_context(
                    tc.tile_pool(name="ps512", bufs=2, space="PSUM"))
                pstr = main.enter_context(
                    tc.tile_pool(name="pstr", bufs=2, space="PSUM"))
                psS = main.enter_context(
                    tc.tile_pool(name="psS", bufs=1, space="PSUM"))
                psA = main.enter_context(
                    tc.tile_pool(name="psA", bufs=2, space="PSUM"))

                kT = {}
                vt = {}

                for qtr in range(4):
                    b, lh = qtr // 2, qtr % 2
                    tq = qtr * 512

                    hid_t = []
                    for ht in range(NHT):
                        t = hid_pool.tile([128, 512], F32, tag="hid")
                        dma.dma_start(
                            t[:], hidT[ht * 128:(ht + 1) * 128, tq:tq + 512])
                        hid_t.append(t)

                    if lh == 0:
                        for h in range(NHEADS):
                            kT[(b, h)] = kt_pool.tile([128, SEQ], F32,
                                                      name="kTt", tag=f"kT{h}")
                        for kt in range(8):
                            vt[(b, kt)] = v_pool.tile([128, NHEADS * 128], F32,
                                                      name="vtt", tag=f"v{kt}")

                    qT = {}
                    for ft in range(NFT):
                        ps = ps512.tile([128, 512], F32, tag="ps512")
                        for hg in range(8):
                            wt = w_pool.tile([128, 640], F32, tag="w")
                            src = wqkvT[hg * 640:(hg + 1) * 640,
                                        ft * 128:(ft + 1) * 128]
                            dma.dma_start(
                                wt[:].rearrange("p (g c) -> p g c", c=128),
                                src.rearrange("(g p) c -> p g c", p=128))
                            for hl in range(5):
                                ht = hg * 5 + hl
                                nc.tensor.matmul(
                                    ps[:], wt[:, hl * 128:(hl + 1) * 128],
                                    hid_t[ht][:],
                                    start=(ht == 0), stop=(ht == NHT - 1))
                        if ft < 5:
                            h = ft
                            q = qt_pool.tile([128, 512], F32, tag=f"qT{h}")
                            nc.vector.tensor_copy(q[:], ps[:])
                            qT[h] = q
                        elif ft < 10:
                            h = ft - 5
                            nc.vector.tensor_copy(
                                kT[(b, h)][:, lh * 512:(lh + 1) * 512], ps[:])
                        else:
                            h = ft - 10
                            vtt = vt_pool.tile([128, 512], F32, tag="vT")
                            nc.vector.tensor_copy(vtt[:], ps[:])
                            for i in range(4):
                                ktl = lh * 4 + i
                                pt = pstr.tile([128, 128], F32, tag="pstr")
                                nc.tensor.transpose(
                                    pt[:], vtt[:, i * 128:(i + 1) * 128],
                                    ident[:])
                                nc.vector.tensor_copy(
                                    vt[(b, ktl)][:, h * 128:(h + 1) * 128],
                                    pt[:])

                    for h in range(NHEADS):
                        eb = sml_pool.tile([128, 8], F32, tag="eb")
                        dma.dma_start(eb[:], expb[h])
                        sj = None
                        if lh == 1:
                            sj = msk_pool.tile([128, 512], F32, tag="sj")
                            dma.dma_start(sj[:], slopej[h])
                        for i in range(4):
                            qtg = lh * 4 + i
                            T = (qtg + 1) * 128
                            nfull = (qtg * 128) // 512
                            off = nfull * 512
                            W = T - off

                            rm = msk_pool.tile([128, 512], F32, tag="rm")
                            dma.dma_start(rm[:, 0:W], rowmask[h, qtg, :, 0:W])

                            psc = psS.tile([128, 1024], F32, tag="psS")
                            if nfull:
                                nc.tensor.matmul(
                                    psc[:, 0:512],
                                    qT[h][:, i * 128:(i + 1) * 128],
                                    kT[(b, h)][:, 0:512],
                                    start=True, stop=True)
                            nc.tensor.matmul(
                                psc[:, off:off + W],
                                qT[h][:, i * 128:(i + 1) * 128],
                                kT[(b, h)][:, off:off + W],
                                start=True, stop=True)

                            sp = sp_pool.tile([128, 1024], F32, tag="sP")
                            if nfull:
                                nc.vector.tensor_add(
                                    sp[:, 0:512], psc[:, 0:512], sj[:])
                            nc.vector.tensor_add(
                                sp[:, off:off + W], psc[:, off:off + W],
                                rm[:, 0:W])

                            rs = sml_pool.tile([128, 1], F32, tag="rs")
                            nc.scalar.activation(
                                sp[:, 0:T], sp[:, 0:T],
                                mybir.ActivationFunctionType.Exp,
                                bias=eb[:, qtg:qtg + 1], scale=1.0,
                                accum_out=rs[:])
                            rc = sml_pool.tile([128, 1], F32, tag="rc")
                            nc.vector.reciprocal(rc[:], rs[:])
                            nc.vector.tensor_scalar_mul(
                                sp[:, 0:T], sp[:, 0:T], rc[:])

                            ptt = pt_pool.tile([128, 1024], F32, tag="pT")
                            for kt in range(qtg + 1):
                                pp = pstr.tile([128, 128], F32, tag="pstr")
                                nc.tensor.transpose(
                                    pp[:], sp[:, kt * 128:(kt + 1) * 128],
                                    ident[:])
                                nc.vector.tensor_copy(
                                    ptt[:, kt * 128:(kt + 1) * 128], pp[:])

                            pa = psA.tile([128, 128], F32, tag="psA")
                            for kt in range(qtg + 1):
                                nc.tensor.matmul(
                                    pa[:],
                                    vt[(b, kt)][:, h * 128:(h + 1) * 128],
                                    ptt[:, kt * 128:(kt + 1) * 128],
                                    start=(kt == 0), stop=(kt == qtg))
                            ats = ats_pool.tile([128, 128], F32, tag="ats")
                            nc.vector.tensor_copy(ats[:], pa[:])
                            dma.dma_start(
                                attnT_d[h * 128:(h + 1) * 128,
                                        b * SEQ + qtg * 128:
                                        b * SEQ + (qtg + 1) * 128],
                                ats[:])

            with ExitStack() as tail:
                at_pool = tail.enter_context(tc.tile_pool(name="aT", bufs=6))
                ot_pool = tail.enter_context(tc.tile_pool(name="oT", bufs=7))
                oe_pool = tail.enter_context(tc.tile_pool(name="oev", bufs=3))
                pso = tail.enter_context(
                    tc.tile_pool(name="psO", bufs=2, space="PSUM"))

                for tch in range(2):
                    aT = []
                    for fh in range(NHEADS):
                        a = at_pool.tile([128, 1024], F32, tag=f"aT{fh}")
                        dma.dma_start(
                            a[:], attnT_d[fh * 128:(fh + 1) * 128,
                                          tch * 1024:(tch + 1) * 1024])
                        aT.append(a)
                    for oc in range(10):
                        ott = []
                        for fh in range(NHEADS):
                            o = ot_pool.tile([128, 512], F32, tag="oT")
                            dma.dma_start(
                                o[:], oT[fh * 128:(fh + 1) * 128,
                                         oc * 512:(oc + 1) * 512])
                            ott.append(o)
                        for tt in range(8):
                            ps = pso.tile([128, 512], F32, tag="psO")
                            for fh in range(NHEADS):
                                nc.tensor.matmul(
                                    ps[:], aT[fh][:, tt * 128:(tt + 1) * 128],
                                    ott[fh][:],
                                    start=(fh == 0), stop=(fh == NHEADS - 1))
                            oe = oe_pool.tile([128, 512], F32, tag="oev")
                            nc.vector.tensor_copy(oe[:], ps[:])
                            dma.dma_start(
                                out[tch * 1024 + tt * 128:
                                    tch * 1024 + (tt + 1) * 128,
                                    oc * 512:(oc + 1) * 512],
                                oe[:])

    split_multi_waits(nc)
    return nc


_NC = None


def _get_nc():
    global _NC
    if _NC is None:
        _NC = build_nc()
    return _NC


_EXEC = None


def _get_exec():
    global _EXEC
    if _EXEC is None:
        import jax
        import jax.numpy as jnp
        from jax.sharding import Mesh, PartitionSpec, NamedSharding
        from jax.experimental.shard_map import shard_map
        from concourse import bass2jax

        nc = _get_nc()
        bass2jax.install_neuronx_cc_hook()

        part_name = (nc.partition_id_tensor.name
                     if nc.partition_id_tensor else None)
        in_names, out_names, out_avals, zero_shapes = [], [], [], []
        for alloc in nc.m.functions[0].allocations:
            if not isinstance(alloc, mybir.MemoryLocationSet):
                continue
            name = alloc.memorylocations[0].name
            if alloc.kind == "ExternalInput":
                if name != part_name:
                    in_names.append(name)
            elif alloc.kind == "ExternalOutput":
                out_names.append(name)
                shape = tuple(alloc.tensor_shape)
                dtype = mybir.dt.np(alloc.dtype)
                out_avals.append(jax.core.ShapedArray(shape, dtype))
                zero_shapes.append((shape, dtype))
        n_params = len(in_names)
        all_names = in_names + out_names
        if part_name is not None:
            all_names = all_names + [part_name]

        def _body(*args):
            operands = list(args)
            if part_name is not None:
                operands.append(bass2jax.partition_id_tensor())
            outs = bass2jax._bass_exec_p.bind(
                *operands,
                out_avals=tuple(out_avals),
                in_names=tuple(all_names),
                out_names=tuple(out_names),
                lowering_input_output_aliases=(),
                sim_require_finite=True,
                sim_require_nnan=True,
                nc=nc,
            )
            return tuple(outs)

        devices = jax.devices()[:8]
        mesh = Mesh(np.asarray(devices), ("core",))
        donate = tuple(range(n_params, n_params + len(out_names)))
        sharded = jax.jit(
            shard_map(
                _body, mesh=mesh,
                in_specs=(PartitionSpec("core"),) * (n_params + len(out_names)),
                out_specs=(PartitionSpec("core"),) * len(out_names),
                check_rep=False,
            ),
            donate_argnums=donate, keep_unused=True,
        )

        zero_sharding = NamedSharding(mesh, PartitionSpec("core"))

        def make_zeros():
            mk = jax.jit(
                lambda: tuple(
                    jnp.zeros((8 * s[0], *s[1:]), d) for s, d in zero_shapes),
                out_shardings=(zero_sharding,) * len(zero_shapes))
            return mk()

        _EXEC = dict(fn=sharded, in_names=in_names, out_names=out_names,
                     mesh=mesh, in_sharding=zero_sharding,
                     make_zeros=make_zeros, n_params=n_params)
    return _EXEC


def device_inputs(in_maps):
    import jax
    ex = _get_exec()
    concat = [np.concatenate([np.asarray(in_maps[c][n]) for c in range(8)],
                             axis=0) for n in ex["in_names"]]
    return [jax.device_put(a, ex["in_sharding"]) for a in concat]


def run_fast(dev_in):
    import jax
    ex = _get_exec()
    outs = ex["fn"](*dev_in, *ex["make_zeros"]())
    jax.block_until_ready(outs)
    return outs[0]


def prep_inputs(hidden_states, attention_mask, W_pack, o_proj):
    hs = np.asarray(hidden_states, dtype=np.float32)
    mask = np.asarray(attention_mask, dtype=np.float32)
    wp = np.asarray(W_pack, dtype=np.float32)
    op = np.asarray(o_proj, dtype=np.float32)

    hidT = np.ascontiguousarray(
        hs.reshape(TOK, H).T)
    scale = 1.0 / np.sqrt(D)
    in_maps = []
    for c in range(8):
        heads = range(5 * c, 5 * c + 5)
        wcols = []
        for kind, base in (("q", 0), ("k", H), ("v", 2 * H)):
            for g in heads:
                blk = wp[base + g * D: base + (g + 1) * D, :]
                if kind == "q":
                    blk = blk * scale
                wcols.append(blk.T)
        wqkvT = np.ascontiguousarray(np.concatenate(wcols, axis=1))
        oTc = np.ascontiguousarray(op[:, 640 * c: 640 * (c + 1)].T)

        sj = np.empty((NHEADS, 128, 512), dtype=np.float32)
        rmk = np.full((NHEADS, 8, 128, 512), -1e30, dtype=np.float32)
        eb = np.empty((NHEADS, 128, 8), dtype=np.float32)
        for hl, g in enumerate(heads):
            m = mask[g]
            sj[hl, :, :] = m[1023, :512][None, :]
            diag = np.diagonal(m).astype(np.float32)
            for qt in range(8):
                Wd = (qt % 4 + 1) * 128
                off = (qt // 4) * 512
                rmk[hl, qt, :, :Wd] = np.maximum(
                    m[qt * 128:(qt + 1) * 128, off:off + Wd], -1e30)
                eb[hl, :, qt] = -(diag[qt * 128:(qt + 1) * 128] + 30.0)
        in_maps.append({
            "hidT": hidT, "wqkvT": wqkvT, "oT": oTc,
            "slopej": sj, "rowmask": rmk, "expb": eb,
        })
    return in_maps


def kernel(hidden_states, attention_mask, W_pack, o_proj):
    in_maps = prep_inputs(hidden_states, attention_mask, W_pack, o_proj)
    try:
        dev_in = device_inputs(in_maps)
        parts = np.asarray(run_fast(dev_in))
        total = parts.reshape(8, TOK, H).sum(axis=0, dtype=np.float32)
    except Exception:
        nc = _get_nc()
        res = run_bass_kernel_spmd(nc, in_maps, core_ids=list(range(8)))
        total = np.zeros((TOK, H), dtype=np.float32)
        for c in range(8):
            total += res.results[c]["out"]
    return total.reshape(NB, SEQ, H)


if __name__ == "__main__":
    t0 = time.time()
    nc = _get_nc()
    print("build+schedule ok in", time.time() - t0, "s")
